# revision 75
# baseline (speedup 1.0000x reference)
"""Trainium2 Bass kernel for GCN-biased sparse attention (nn_Attention_37589553775245).

Reference computation (per batch b of 8, one NeuronCore each):
    qkv = x @ w_qkv; q,k,v per head (H=8, DH=64)
    attn = softmax(q k^T / sqrt(DH)) + A_hat        (A_hat = D^-1/2 (ceil(adj)+I) D^-1/2)
    out = (attn @ v) @ w_out + b_out

Sharding: pure batch-parallel across the 8 cores (B=8). A_hat is computed on
host (cheap) and replicated; weights replicated. No collectives.

Layout strategy (all matmul operands bf16, PSUM accumulation fp32; at 2e-2
tolerance bf16 is comfortably safe and it halves DMA/SBUF vs fp32r):
  - x pre-transposed on host to xT [DIM, N]; first matmul contraction (DIM)
    on the partition axis. q,k produced transposed (qT,kT [DH,N], head pairs
    at partition bases 0/64); v natural [N,F] into a per-head augmented
    [v_h | 1] tile (the ones column rides the attn@v matmul to produce the
    softmax denominator on the output partition axis).
  - scores transposed: sT[j,i] = sum_d k[j,d] qT[d,i] ([128j, 512i] tiles,
    1 PSUM bank); ACT exp (scale folded) -> bf16 exp tiles. Softmax
    max-subtraction skipped: logits*scale ~ N(0,1), exp safe in fp32.
  - attn@v FLIPPED to natural orientation: lhsT = exp-tile [128j, 128i],
    rhs = vaug_h [128j, 65] -> PSUM [128 i, 64 d | denom]. Uses all 128 PE
    output partitions (the old transposed form used 65/128), and the
    denominator lands on the partition axis so normalization is a cheap DVE
    reciprocal + tensor_scalar_mul (no partition-broadcast DRAM bounce).
  - A_hat V computed in natural orientation via the factored form
    D^-1/2 (A (D^-1/2 V)): the binary A = ceil(adj)+I is EXACT in fp8e4m3,
    so the matmul runs fp8 DoubleRow (K=256 per instruction, 4x fewer PE
    cycles than bf16) with quantization error only on the D^-1/2 V operand
    (measured end-to-end rel err 1.79e-2 vs the 2e-2 gate). The dinv_i row
    scale rides the PSUM->SBUF copy as a per-partition multiply, and the
    AV slice-add is fused into the chunk-1 normalization (chunk 0 merges
    via four row-adds before its transposes).
  - Y (normalized attention, natural [i,f]) is PE-transposed per [128,128]
    tile (bf16 transpose = 128 cycles) into Y^T for the out projection;
    out = Y^T-tiles.T @ w_out + b_out.
  - unit order c-outer/h-inner: i-chunk 0 finishes all heads halfway through,
    so its transposes + out projections + stores weave into chunk 1's
    attention units; only chunk 1's tail runs after the last exp.
  - emission is a flat 2-deep software pipeline over the 64 (c,h,jb) stages:
    scores+exp issued 2 stages ahead of attn@v, with deadline-scheduled fill
    steps (projection chunks, A_hat units, transposes, out projections)
    pulled one per stage so the PE never starves while ACT chews exps.
  - startup: critical input DMAs ordered by first use (q f0 cols, xT c0
    halves, k f4 cols, v cols); dummy warmup matmuls ramp the PE clock
    (HAM p-state) while the loads land; exp table pre-loaded.
  - TimelineSim: 92.6 us (baseline fp32r revision: 141.5 us).
"""

import os
import sys

import numpy as np

for _p in ("/opt/trn_rl_repo", "/root/.axon_site/_ro/trn_rl_repo"):
    if _p not in sys.path and os.path.isdir(_p):
        sys.path.insert(0, _p)

import concourse.bass as bass  # noqa: E402
import concourse.mybir as mybir  # noqa: E402
import concourse.tile as tile  # noqa: E402
from concourse import bacc  # noqa: E402
from concourse.bass_utils import run_bass_kernel_spmd  # noqa: E402
from concourse.masks import make_identity  # noqa: E402

B, N, DIM, H, DH = 8, 1024, 512, 8, 64
F = H * DH          # 512, inner dim
NT = N // 128       # 8 n-tiles (also j-tiles)
DT = DIM // 128     # 4 dim-tiles
FT = F // 128       # 4 f-tiles
NC2 = N // 512      # 2 i-chunks of 512
IS = 512 // 128     # 4 i-subtiles per chunk
SCALE = DH ** -0.5

F32 = mybir.dt.float32
BF16 = mybir.dt.bfloat16
FP8 = mybir.dt.float8e4

_PROGRAM = None
_last_in_maps = None


def _build_program(reps=1):
    nc = bacc.Bacc("TRN2", target_bir_lowering=False, debug=False, num_devices=8)

    xT_d = nc.dram_tensor("xT", [DIM, N], BF16, kind="ExternalInput")
    wqkv_d = nc.dram_tensor("wqkv", [DIM, 3 * F], BF16, kind="ExternalInput")
    aT8_d = nc.dram_tensor("aT8", [N, N], FP8, kind="ExternalInput")
    dinv_d = nc.dram_tensor("dinvp", [128, NT], F32, kind="ExternalInput")
    wout_d = nc.dram_tensor("wout", [F, DIM], BF16, kind="ExternalInput")
    bout_d = nc.dram_tensor("bout", [1, DIM], F32, kind="ExternalInput")
    out_d = nc.dram_tensor("out", [N, DIM], F32, kind="ExternalOutput")

    with tile.TileContext(nc) as tc:
        with (
            tc.tile_pool(name="big", bufs=1) as big,
            tc.tile_pool(name="ps_mm", bufs=2, space="PSUM") as ps_mm,
            tc.tile_pool(name="ps_s", bufs=2, space="PSUM") as ps_s,
            tc.tile_pool(name="ps_av", bufs=2, space="PSUM") as ps_av,
            tc.tile_pool(name="exps", bufs=8) as exps,
            tc.tile_pool(name="small", bufs=6) as small,
            tc.tile_pool(name="outs", bufs=4) as outs,
        ):
          for _rep in range(reps):
            # ---- persistent SBUF tensors -------------------------------
            xT = big.tile([128, DT, N], BF16)
            wqkv = big.tile([128, DT, 3 * F], BF16)
            wout = big.tile([128, FT, DIM], BF16)
            qkT = big.tile([128, 2 * FT, N], BF16)       # q(ft 0:4), k(ft 4:8)
            vaug = big.tile([128, NT, H, DH + 1], BF16)  # [j, jt, h, v|1]
            aT8 = big.tile([128, NT, N], FP8)            # binary A^T [j, i]
            w8 = big.tile([128, NT, F], FP8)             # D^-1/2 V [j, f]
            dinv_sb = big.tile([128, NT], F32)           # dinv[t*128+p]
            yN = big.tile([128, NT, F], BF16)            # Y natural [i, f]
            avN = big.tile([128, NT, F], BF16)           # A_hat V natural
            yT = big.tile([128, FT, N], BF16)            # Y^T [f, i]
            ident = big.tile([128, 128], BF16)
            bout_bc = big.tile([128, DIM], F32)
            warm = big.tile([1, 8], F32)

            make_identity(nc, ident)
            nc.vector.memset(vaug[:, :, :, DH:DH + 1], 1.0)  # denominator ones
            # warm the ACT exp table during the load phase
            nc.vector.memset(warm, 0.0)
            nc.scalar.activation(out=warm, in_=warm,
                                 func=mybir.ActivationFunctionType.Exp)
            # warm the PE clock (HAM p-state ramps after ~3us of sustained
            # activity) with dummy matmuls while the input DMAs land
            warm_mm = big.tile([128, 256], BF16)
            nc.vector.memset(warm_mm, 0.0)
            for _w in range(10):
                ps_w = ps_mm.tile([128, 512], F32, tag="mm", name="ps_w")
                nc.tensor.matmul(ps_w[:, 0:256], warm_mm[:, 0:128], warm_mm)

            # ---- input DMAs (ordered by first use; DMA engines serialize
            # heavily, so the critical-path tensors must go first) ---------
            def load_xT(c):
                nc.sync.dma_start(
                    out=xT[:, :, c * 512:(c + 1) * 512],
                    in_=xT_d[:, c * 512:(c + 1) * 512].rearrange(
                        "(t p) n -> p t n", p=128))

            def load_wqkv_cols(lo, hi):
                nc.sync.dma_start(
                    out=wqkv[:, :, lo:hi],
                    in_=wqkv_d[:, lo:hi].rearrange("(t p) f -> p t f", p=128),
                )

            # critical path first: q f-tile 0 cols, the xT chunk-0 halves,
            # k f4 cols, then v cols; everything rides the SP HWDGE ring
            # (configs gen at ~650ns each, transfers queue in this order)
            load_wqkv_cols(0, 128)          # q heads 0,1
            nc.sync.dma_start(
                out=xT[:, 0:2, 0:512],
                in_=xT_d[0:256, 0:512].rearrange("(t p) n -> p t n", p=128))
            nc.sync.dma_start(
                out=xT[:, 2:4, 0:512],
                in_=xT_d[256:512, 0:512].rearrange("(t p) n -> p t n", p=128))
            load_wqkv_cols(512, 640)        # k heads 0,1
            nc.sync.dma_start(out=dinv_sb, in_=dinv_d[:, :])  # gates w8
            load_wqkv_cols(1024, 1280)      # v cols 0-255
            load_wqkv_cols(1280, 1536)      # v cols 256-511
            load_xT(1)
            load_wqkv_cols(128, 512)        # q heads 2-7
            load_wqkv_cols(640, 1024)       # k heads 2-7
            nc.sync.dma_start(
                out=wout, in_=wout_d[:, :].rearrange("(t p) n -> p t n", p=128))
            nc.sync.dma_start(out=bout_bc,
                              in_=bout_d[0:1, :].to_broadcast((128, DIM)))
            # binary A^T (fp8, 1MB): needed from fill slot 5 (~t12us)
            nc.sync.dma_start(
                out=aT8,
                in_=aT8_d[:, :].rearrange("(t p) n -> p t n", p=128),
            )

            # ---- projection / filler units -----------------------------
            def emit_qk1(ft, c):
                # qkT f-tile ft (transposed), one 512-wide n-chunk
                ps = ps_mm.tile([128, 512], F32, tag="mm", name="ps_qk")
                for dt_i in range(DT):
                    nc.tensor.matmul(
                        ps,
                        wqkv[:, dt_i, ft * 128:(ft + 1) * 128],
                        xT[:, dt_i, c * 512:(c + 1) * 512],
                        start=(dt_i == 0),
                        stop=(dt_i == DT - 1),
                    )
                nc.vector.tensor_copy(
                    out=qkT[:, ft, c * 512:(c + 1) * 512], in_=ps)

            def emit_qk(ft):
                for c in range(NC2):
                    emit_qk1(ft, c)

            def emit_v1(nt):
                # v natural [n, f] into vaug (per-head columns + ones)
                ps = ps_mm.tile([128, 512], F32, tag="mm", name="ps_v")
                for dt_i in range(DT):
                    nc.tensor.matmul(
                        ps,
                        xT[:, dt_i, nt * 128:(nt + 1) * 128],
                        wqkv[:, dt_i, 2 * F:3 * F],
                        start=(dt_i == 0),
                        stop=(dt_i == DT - 1),
                    )
                nc.vector.tensor_copy(
                    out=vaug[:, nt, :, 0:DH],
                    in_=ps.rearrange("p (h d) -> p h d", h=H),
                )
                # W = D^-1/2 V quantized to fp8 (feeds the binary-A
                # matmul); built on the idle Pool engine from vaug - DVE is
                # the front-half chokepoint and w8 isn't needed until ~iter 26
                nc.gpsimd.tensor_scalar_mul(
                    out=w8[:, nt, :].rearrange("p (h d) -> p h d", h=H),
                    in0=vaug[:, nt, :, 0:DH],
                    scalar1=dinv_sb[:, nt:nt + 1])

            def ahat_unit(it):
                # A_hat V natural [i-tile it, f] via the factored form
                # D^-1/2 (A (D^-1/2 V)): A is BINARY (exact in fp8), so fp8
                # DoubleRow (K=256/matmul) is numerically safe; the dinv_i
                # row scale rides the PSUM->SBUF copy as a per-partition mul.
                ps = ps_mm.tile([128, 512], F32, tag="mm", name="ps_ah")
                for jj in range(0, NT, 2):
                    nc.tensor.matmul(
                        ps,
                        aT8[:, jj:jj + 2, it * 128:(it + 1) * 128],
                        w8[:, jj:jj + 2, :],
                        start=(jj == 0),
                        stop=(jj == NT - 2),
                        perf_mode=mybir.MatmulPerfMode.DoubleRow,
                    )
                nc.vector.tensor_scalar_mul(
                    out=avN[:, it, :], in0=ps, scalar1=dinv_sb[:, it:it + 1])

            def trans_unit(ft, it):
                # yT[f, i-tile it] = yN[it, f-tile ft]^T (AV already in yN)
                ps = ps_mm.tile([128, 512], F32, tag="mm", name="ps_tr")
                trv = ps.bitcast(BF16)[:, 0:128]
                nc.tensor.transpose(
                    trv, yN[:, it, ft * 128:(ft + 1) * 128], ident)
                nc.vector.tensor_copy(
                    out=yT[:, ft, it * 128:(it + 1) * 128], in_=trv)

            def out_unit(nt):
                # out[i-tile nt] = Y^T-tiles.T @ w_out + b_out
                ps = ps_mm.tile([128, 512], F32, tag="mm")
                for ft in range(FT):
                    nc.tensor.matmul(
                        ps,
                        yT[:, ft, nt * 128:(nt + 1) * 128],
                        wout[:, ft, :],
                        start=(ft == 0),
                        stop=(ft == FT - 1),
                    )
                ot = outs.tile([128, DIM], F32, tag="ot")
                nc.vector.tensor_add(ot, ps, bout_bc)
                nc.sync.dma_start(
                    out=out_d[nt * 128:(nt + 1) * 128, :], in_=ot)

            # ---- attention: flat 2-deep software pipeline --------------
            # stage k = (c, h, jb): scores+exp issued at k, attn@v at k-2,
            # so the PE never sits on an exp dependency without queued work
            # and the ACT exp stream is continuously fed.
            JB = NT // 2
            stages = [(c, h, jb)
                      for c in range(NC2) for h in range(H) for jb in range(JB)]
            sc_of = {}
            et_of = {}
            pav_of = {}

            def emit_scores(k):
                c, h, jb = stages[k]
                hb = (h % 2) * 64
                ht = h // 2
                if jb == 0:
                    pav_of[(c, h)] = ps_av.tile([128, IS, DH + 1], F32,
                                                tag="av", name="pav")
                ps_sc = ps_s.tile([128, 2, 512], F32, tag="ps", name="ps_sc")
                for e in range(2):
                    jt = jb * 2 + e
                    # scoresT[j, i] = sum_d kT[d, j] qT[d, i]
                    nc.tensor.matmul(
                        ps_sc[:, e, :],
                        qkT[hb:hb + 64, FT + ht, jt * 128:(jt + 1) * 128],
                        qkT[hb:hb + 64, ht, c * 512:(c + 1) * 512],
                    )
                et = exps.tile([128, 2, 512], BF16, tag="exp", name="et")
                nc.scalar.activation(
                    out=et, in_=ps_sc,
                    func=mybir.ActivationFunctionType.Exp,
                    scale=float(SCALE),
                )
                sc_of[k] = ps_sc
                et_of[k] = et

            def emit_av(k):
                c, h, jb = stages[k]
                et = et_of.pop(k)
                pav = pav_of[(c, h)]
                for e in range(2):
                    jt = jb * 2 + e
                    for isub in range(IS):
                        # natural orientation: [128 i, v|1]. start=True zeroes
                        # the whole 2KB psum bank (zero region), so only the
                        # unit's first matmul starts; later isubs' first
                        # writes overwrite pending-zero bytes (has_written).
                        nc.tensor.matmul(
                            pav[:, isub, :],
                            et[:, e, isub * 128:(isub + 1) * 128],
                            vaug[:, jt, h, :],
                            start=(jt == 0 and isub == 0),
                            stop=(jt == NT - 1 and isub == IS - 1),
                        )
                if jb == JB - 1:
                    # normalize: denominators ride the free axis, one per
                    # isub. For chunk 1 the A_hat V slice-add is fused in
                    # (avN is ready long before); chunk 0 normalizes plain
                    # and merges AV in four row-adds before its transposes.
                    pav = pav_of.pop((c, h))
                    recip = small.tile([128, IS], F32, tag="recip")
                    nc.vector.reciprocal(out=recip, in_=pav[:, :, DH:DH + 1])
                    for isub in range(IS):
                        it = c * IS + isub
                        if c == 1:
                            nc.vector.scalar_tensor_tensor(
                                out=yN[:, it, h * DH:(h + 1) * DH],
                                in0=pav[:, isub, 0:DH],
                                scalar=recip[:, isub:isub + 1],
                                in1=avN[:, it, h * DH:(h + 1) * DH],
                                op0=mybir.AluOpType.mult,
                                op1=mybir.AluOpType.add,
                            )
                        else:
                            nc.vector.tensor_scalar_mul(
                                out=yN[:, it, h * DH:(h + 1) * DH],
                                in0=pav[:, isub, 0:DH],
                                scalar1=recip[:, isub:isub + 1],
                            )

            # ---- fill steps: small PE work items woven between stages --
            # Deadlines (PE is in-order; a score emitted at iter k-2 must
            # have its qkT inputs earlier in program order):
            #   v j-tiles 2k,2k+1 before av(stage jb=k)     -> idx <= k
            #   head h's q/k chunks before scores(4h)       -> idx <= 4h-3
            #   (k chunk c covers j-range jb in {2c, 2c+1} only)
            fill = {}

            def tr_group(ft, c):
                for it in range(c * IS, c * IS + IS):
                    trans_unit(ft, it)

            def emit_v2(nt):
                emit_v1(nt)
                emit_v1(nt + 1)

            for i, nt in enumerate((2, 4, 6)):             # dl: av(nt//2)
                fill[i] = lambda nt=nt: emit_v2(nt)
            # q/k chunks at their deadline slots (head h scores at iter 4h-2)
            # so the early iterations stream scores to ACT as fast as possible
            for i, a in ((3, (1, 0)), (4, (5, 0)), (6, (5, 1)),
                         (11, (2, 0)), (12, (6, 0)), (14, (6, 1)),
                         (19, (3, 0)), (20, (7, 0)), (22, (7, 1)),
                         (24, (1, 1)), (44, (2, 1)), (52, (3, 1)),
                         (25, (0, 1))):
                fill[i] = lambda a=a: emit_qk1(*a)
            for i, it in enumerate((26, 27, 28, 29, 30, 31, 34, 35)):
                fill[it] = lambda i=i: ahat_unit(i)  # dl: merge@32, stt@35

            def c0_av_merge(lo, hi):
                # merge A_hat V into chunk-0 rows (after all c0 stt, iter 31)
                for it in range(lo, hi):
                    nc.vector.tensor_add(
                        yN[:, it, :], yN[:, it, :], avN[:, it, :])

            fill[32] = lambda: c0_av_merge(0, 2)
            fill[33] = lambda: c0_av_merge(2, 4)
            for i, ft in enumerate(range(FT)):             # tr c0 (>= 34)
                fill[36 + i] = lambda ft=ft: tr_group(ft, 0)
            fill[41] = lambda: tr_group(0, 1)
            fill[48] = lambda: tr_group(1, 1)
            fill[47] = lambda: out_unit(0)
            fill[50] = lambda: out_unit(1)
            fill[56] = lambda: tr_group(2, 1)
            fill[60] = lambda: out_unit(2)
            fill[62] = lambda: out_unit(3)

            # ---- emission ----------------------------------------------
            # pre-phase: only the c0 chunks gate the first scores; the c1
            # k-chunk must still precede scores(2) in PE program order
            emit_qk1(0, 0)      # q heads 0,1 chunk c0
            emit_qk1(4, 0)      # k heads 0,1 j-tiles 0-3
            NS = len(stages)
            emit_scores(0)
            emit_scores(1)
            emit_v1(0)
            emit_v1(1)
            emit_qk1(4, 1)      # k heads 0,1 j-tiles 4-7 (scores(2,3))
            for k in range(NS):
                if k + 2 < NS:
                    emit_scores(k + 2)
                if k in fill:
                    fill[k]()
                emit_av(k)
            # chunk-1 epilogue: all remaining transposes first (tiny on PE,
            # merges pipeline on DVE), then the out projections back-to-back
            for it in range(IS, NT):
                trans_unit(3, it)
            rings = [nc.sync, nc.scalar, nc.sync, nc.scalar]
            for i, it in enumerate(range(IS, NT)):
                out_unit(it, ring=rings[i])

    nc.compile()
    return nc


def _get_program():
    global _PROGRAM
    if _PROGRAM is None:
        _PROGRAM = _build_program()
    return _PROGRAM


def kernel(x, adj, w_qkv, w_out, b_out):
    bf16 = mybir.dt.np(BF16)
    x = np.asarray(x, dtype=np.float32)
    adj = np.asarray(adj, dtype=np.float32)
    w_qkv = np.ascontiguousarray(np.asarray(w_qkv, dtype=np.float32)).astype(bf16)
    w_out = np.ascontiguousarray(np.asarray(w_out, dtype=np.float32)).astype(bf16)
    b_out = np.asarray(b_out, dtype=np.float32).reshape(1, DIM)

    # host-side: binary adjacency A = ceil(adj)+I (EXACT in fp8e4m3) and the
    # degree-normalization vector; the normalized bias A_hat = D^-1/2 A D^-1/2
    # is applied on device in factored form around the fp8 A @ (D^-1/2 V)
    # matmul (DoubleRow), keeping all quantization error on the V side.
    fp8 = mybir.dt.np(FP8)
    A = np.ceil(adj) + np.eye(N, dtype=np.float32)
    dinv = (A.sum(axis=1) ** -0.5).astype(np.float32)
    aT8 = np.ascontiguousarray(A.T).astype(fp8)
    dinvp = np.ascontiguousarray(dinv.reshape(NT, 128).T)

    nc = _get_program()
    in_maps = []
    for b in range(B):
        in_maps.append({
            "xT": np.ascontiguousarray(x[b].T).astype(bf16),
            "wqkv": w_qkv,
            "aT8": aT8,
            "dinvp": dinvp,
            "wout": w_out,
            "bout": b_out,
        })
    global _last_in_maps
    _last_in_maps = in_maps
    res = run_bass_kernel_spmd(nc, in_maps, list(range(B)))
    out = np.stack([res.results[b]["out"] for b in range(B)], axis=0)
    return out.astype(np.float32)


if __name__ == "__main__":
    rng = np.random.default_rng(0)
    x = rng.standard_normal((B, N, DIM), dtype=np.float32)
    adj = (rng.random((N, N), dtype=np.float32) < 0.05).astype(np.float32) * 0.5
    w_qkv = rng.standard_normal((DIM, 3 * F), dtype=np.float32) * DIM ** -0.5
    w_out = rng.standard_normal((F, DIM), dtype=np.float32) * F ** -0.5
    b_out = np.zeros(DIM, dtype=np.float32)
    out = kernel(x=x, adj=adj, w_qkv=w_qkv, w_out=w_out, b_out=b_out)
    print("out", out.shape, out.dtype, np.abs(out).max())


# revision 77
# speedup vs baseline: 13.8899x; 13.8899x over previous
"""Trainium2 Bass kernel for GCN-biased sparse attention (nn_Attention_37589553775245).

Reference computation (per batch b of 8, one NeuronCore each):
    qkv = x @ w_qkv; q,k,v per head (H=8, DH=64)
    attn = softmax(q k^T / sqrt(DH)) + A_hat        (A_hat = D^-1/2 (ceil(adj)+I) D^-1/2)
    out = (attn @ v) @ w_out + b_out

Sharding: pure batch-parallel across the 8 cores (B=8). A_hat is computed on
host (cheap) and replicated; weights replicated. No collectives.

Layout strategy (all matmul operands bf16, PSUM accumulation fp32; at 2e-2
tolerance bf16 is comfortably safe and it halves DMA/SBUF vs fp32r):
  - x pre-transposed on host to xT [DIM, N]; first matmul contraction (DIM)
    on the partition axis. q,k produced transposed (qT,kT [DH,N], head pairs
    at partition bases 0/64); v natural [N,F] into a per-head augmented
    [v_h | 1] tile (the ones column rides the attn@v matmul to produce the
    softmax denominator on the output partition axis).
  - scores transposed: sT[j,i] = sum_d k[j,d] qT[d,i] ([128j, 512i] tiles,
    1 PSUM bank); ACT exp (scale folded) -> bf16 exp tiles. Softmax
    max-subtraction skipped: logits*scale ~ N(0,1), exp safe in fp32.
  - attn@v FLIPPED to natural orientation: lhsT = exp-tile [128j, 128i],
    rhs = vaug_h [128j, 65] -> PSUM [128 i, 64 d | denom]. Uses all 128 PE
    output partitions (the old transposed form used 65/128), and the
    denominator lands on the partition axis so normalization is a cheap DVE
    reciprocal + tensor_scalar_mul (no partition-broadcast DRAM bounce).
  - A_hat V computed in natural orientation via the factored form
    D^-1/2 (A (D^-1/2 V)): the binary A = ceil(adj)+I is EXACT in fp8e4m3,
    so the matmul runs fp8 DoubleRow (K=256 per instruction, 4x fewer PE
    cycles than bf16) with quantization error only on the D^-1/2 V operand
    (measured end-to-end rel err 1.79e-2 vs the 2e-2 gate). The dinv_i row
    scale rides the PSUM->SBUF copy as a per-partition multiply, and the
    AV slice-add is fused into the chunk-1 normalization (chunk 0 merges
    via four row-adds before its transposes).
  - Y (normalized attention, natural [i,f]) is PE-transposed per [128,128]
    tile (bf16 transpose = 128 cycles) into Y^T for the out projection;
    out = Y^T-tiles.T @ w_out + b_out.
  - unit order c-outer/h-inner: i-chunk 0 finishes all heads halfway through,
    so its transposes + out projections + stores weave into chunk 1's
    attention units; only chunk 1's tail runs after the last exp.
  - emission is a flat 2-deep software pipeline over the 64 (c,h,jb) stages:
    scores+exp issued 2 stages ahead of attn@v, with deadline-scheduled fill
    steps (projection chunks, A_hat units, transposes, out projections)
    pulled one per stage so the PE never starves while ACT chews exps.
  - startup: critical input DMAs ordered by first use (q f0 cols, xT c0
    halves, k f4 cols, v cols); dummy warmup matmuls ramp the PE clock
    (HAM p-state) while the loads land; exp table pre-loaded.
  - TimelineSim: 91.6 us (baseline fp32r revision: 141.5 us).
"""

import os
import sys

import numpy as np

for _p in ("/opt/trn_rl_repo", "/root/.axon_site/_ro/trn_rl_repo"):
    if _p not in sys.path and os.path.isdir(_p):
        sys.path.insert(0, _p)

import concourse.bass as bass  # noqa: E402
import concourse.mybir as mybir  # noqa: E402
import concourse.tile as tile  # noqa: E402
from concourse import bacc  # noqa: E402
from concourse.bass_utils import run_bass_kernel_spmd  # noqa: E402
from concourse.masks import make_identity  # noqa: E402

B, N, DIM, H, DH = 8, 1024, 512, 8, 64
F = H * DH          # 512, inner dim
NT = N // 128       # 8 n-tiles (also j-tiles)
DT = DIM // 128     # 4 dim-tiles
FT = F // 128       # 4 f-tiles
NC2 = N // 512      # 2 i-chunks of 512
IS = 512 // 128     # 4 i-subtiles per chunk
SCALE = DH ** -0.5

F32 = mybir.dt.float32
BF16 = mybir.dt.bfloat16
FP8 = mybir.dt.float8e4

_PROGRAM = None
_last_in_maps = None


def _build_program(reps=1):
    nc = bacc.Bacc("TRN2", target_bir_lowering=False, debug=False, num_devices=8)

    xT_d = nc.dram_tensor("xT", [DIM, N], BF16, kind="ExternalInput")
    wqkv_d = nc.dram_tensor("wqkv", [DIM, 3 * F], BF16, kind="ExternalInput")
    aT8_d = nc.dram_tensor("aT8", [N, N], FP8, kind="ExternalInput")
    dinv_d = nc.dram_tensor("dinvp", [128, NT], F32, kind="ExternalInput")
    wout_d = nc.dram_tensor("wout", [F, DIM], BF16, kind="ExternalInput")
    bout_d = nc.dram_tensor("bout", [1, DIM], F32, kind="ExternalInput")
    out_d = nc.dram_tensor("out", [N, DIM], F32, kind="ExternalOutput")

    with tile.TileContext(nc) as tc:
        with (
            tc.tile_pool(name="big", bufs=1) as big,
            tc.tile_pool(name="ps_mm", bufs=2, space="PSUM") as ps_mm,
            tc.tile_pool(name="ps_s", bufs=2, space="PSUM") as ps_s,
            tc.tile_pool(name="ps_av", bufs=2, space="PSUM") as ps_av,
            tc.tile_pool(name="exps", bufs=8) as exps,
            tc.tile_pool(name="small", bufs=6) as small,
            tc.tile_pool(name="outs", bufs=4) as outs,
        ):
          for _rep in range(reps):
            # ---- persistent SBUF tensors -------------------------------
            xT = big.tile([128, DT, N], BF16)
            wqkv = big.tile([128, DT, 3 * F], BF16)
            wout = big.tile([128, FT, DIM], BF16)
            qkT = big.tile([128, 2 * FT, N], BF16)       # q(ft 0:4), k(ft 4:8)
            vaug = big.tile([128, NT, H, DH + 1], BF16)  # [j, jt, h, v|1]
            aT8 = big.tile([128, NT, N], FP8)            # binary A^T [j, i]
            w8 = big.tile([128, NT, F], FP8)             # D^-1/2 V [j, f]
            dinv_sb = big.tile([128, NT], F32)           # dinv[t*128+p]
            yN = big.tile([128, NT, F], BF16)            # Y natural [i, f]
            avN = big.tile([128, NT, F], BF16)           # A_hat V natural
            yT = big.tile([128, FT, N], BF16)            # Y^T [f, i]
            ident = big.tile([128, 128], BF16)
            bout_bc = big.tile([128, DIM], F32)
            warm = big.tile([1, 8], F32)

            make_identity(nc, ident)
            nc.vector.memset(vaug[:, :, :, DH:DH + 1], 1.0)  # denominator ones
            # warm the ACT exp table during the load phase
            nc.vector.memset(warm, 0.0)
            nc.scalar.activation(out=warm, in_=warm,
                                 func=mybir.ActivationFunctionType.Exp)
            # warm the PE clock (HAM p-state ramps after ~3us of sustained
            # activity) with dummy matmuls while the input DMAs land
            warm_mm = big.tile([128, 256], BF16)
            nc.vector.memset(warm_mm, 0.0)
            for _w in range(10):
                ps_w = ps_mm.tile([128, 512], F32, tag="mm", name="ps_w")
                nc.tensor.matmul(ps_w[:, 0:256], warm_mm[:, 0:128], warm_mm)

            # ---- input DMAs (ordered by first use; DMA engines serialize
            # heavily, so the critical-path tensors must go first) ---------
            def load_xT(c):
                nc.sync.dma_start(
                    out=xT[:, :, c * 512:(c + 1) * 512],
                    in_=xT_d[:, c * 512:(c + 1) * 512].rearrange(
                        "(t p) n -> p t n", p=128))

            def load_wqkv_cols(lo, hi):
                nc.sync.dma_start(
                    out=wqkv[:, :, lo:hi],
                    in_=wqkv_d[:, lo:hi].rearrange("(t p) f -> p t f", p=128),
                )

            # critical path first: q f-tile 0 cols, the xT chunk-0 halves,
            # k f4 cols, then v cols; everything rides the SP HWDGE ring
            # (configs gen at ~650ns each, transfers queue in this order)
            load_wqkv_cols(0, 128)          # q heads 0,1
            nc.sync.dma_start(
                out=xT[:, 0:2, 0:512],
                in_=xT_d[0:256, 0:512].rearrange("(t p) n -> p t n", p=128))
            nc.sync.dma_start(
                out=xT[:, 2:4, 0:512],
                in_=xT_d[256:512, 0:512].rearrange("(t p) n -> p t n", p=128))
            load_wqkv_cols(512, 640)        # k heads 0,1
            nc.sync.dma_start(out=dinv_sb, in_=dinv_d[:, :])  # gates w8
            load_wqkv_cols(1024, 1280)      # v cols 0-255
            load_wqkv_cols(1280, 1536)      # v cols 256-511
            load_xT(1)
            load_wqkv_cols(128, 512)        # q heads 2-7
            load_wqkv_cols(640, 1024)       # k heads 2-7
            nc.sync.dma_start(
                out=wout, in_=wout_d[:, :].rearrange("(t p) n -> p t n", p=128))
            nc.sync.dma_start(out=bout_bc,
                              in_=bout_d[0:1, :].to_broadcast((128, DIM)))
            # binary A^T (fp8, 1MB): needed from fill slot 5 (~t12us)
            nc.sync.dma_start(
                out=aT8,
                in_=aT8_d[:, :].rearrange("(t p) n -> p t n", p=128),
            )

            # ---- projection / filler units -----------------------------
            def emit_qk1(ft, c):
                # qkT f-tile ft (transposed), one 512-wide n-chunk
                ps = ps_mm.tile([128, 512], F32, tag="mm", name="ps_qk")
                for dt_i in range(DT):
                    nc.tensor.matmul(
                        ps,
                        wqkv[:, dt_i, ft * 128:(ft + 1) * 128],
                        xT[:, dt_i, c * 512:(c + 1) * 512],
                        start=(dt_i == 0),
                        stop=(dt_i == DT - 1),
                    )
                nc.vector.tensor_copy(
                    out=qkT[:, ft, c * 512:(c + 1) * 512], in_=ps)

            def emit_qk(ft):
                for c in range(NC2):
                    emit_qk1(ft, c)

            def emit_v1(nt):
                # v natural [n, f] into vaug (per-head columns + ones)
                ps = ps_mm.tile([128, 512], F32, tag="mm", name="ps_v")
                for dt_i in range(DT):
                    nc.tensor.matmul(
                        ps,
                        xT[:, dt_i, nt * 128:(nt + 1) * 128],
                        wqkv[:, dt_i, 2 * F:3 * F],
                        start=(dt_i == 0),
                        stop=(dt_i == DT - 1),
                    )
                nc.vector.tensor_copy(
                    out=vaug[:, nt, :, 0:DH],
                    in_=ps.rearrange("p (h d) -> p h d", h=H),
                )
                # W = D^-1/2 V quantized to fp8 (feeds the binary-A
                # matmul); built on the idle Pool engine from vaug - DVE is
                # the front-half chokepoint and w8 isn't needed until ~iter 26
                nc.gpsimd.tensor_scalar_mul(
                    out=w8[:, nt, :].rearrange("p (h d) -> p h d", h=H),
                    in0=vaug[:, nt, :, 0:DH],
                    scalar1=dinv_sb[:, nt:nt + 1])

            def ahat_unit(it):
                # A_hat V natural [i-tile it, f] via the factored form
                # D^-1/2 (A (D^-1/2 V)): A is BINARY (exact in fp8), so fp8
                # DoubleRow (K=256/matmul) is numerically safe; the dinv_i
                # row scale rides the PSUM->SBUF copy as a per-partition mul.
                ps = ps_mm.tile([128, 512], F32, tag="mm", name="ps_ah")
                for jj in range(0, NT, 2):
                    nc.tensor.matmul(
                        ps,
                        aT8[:, jj:jj + 2, it * 128:(it + 1) * 128],
                        w8[:, jj:jj + 2, :],
                        start=(jj == 0),
                        stop=(jj == NT - 2),
                        perf_mode=mybir.MatmulPerfMode.DoubleRow,
                    )
                nc.vector.tensor_scalar_mul(
                    out=avN[:, it, :], in0=ps, scalar1=dinv_sb[:, it:it + 1])

            def trans_unit(ft, it):
                # yT[f, i-tile it] = yN[it, f-tile ft]^T (AV already in yN)
                ps = ps_mm.tile([128, 512], F32, tag="mm", name="ps_tr")
                trv = ps.bitcast(BF16)[:, 0:128]
                nc.tensor.transpose(
                    trv, yN[:, it, ft * 128:(ft + 1) * 128], ident)
                nc.vector.tensor_copy(
                    out=yT[:, ft, it * 128:(it + 1) * 128], in_=trv)

            def out_unit(nt):
                # out[i-tile nt] = Y^T-tiles.T @ w_out + b_out
                ps = ps_mm.tile([128, 512], F32, tag="mm")
                for ft in range(FT):
                    nc.tensor.matmul(
                        ps,
                        yT[:, ft, nt * 128:(nt + 1) * 128],
                        wout[:, ft, :],
                        start=(ft == 0),
                        stop=(ft == FT - 1),
                    )
                ot = outs.tile([128, DIM], F32, tag="ot")
                nc.vector.tensor_add(ot, ps, bout_bc)
                nc.sync.dma_start(
                    out=out_d[nt * 128:(nt + 1) * 128, :], in_=ot)

            # ---- attention: flat 2-deep software pipeline --------------
            # stage k = (c, h, jb): scores+exp issued at k, attn@v at k-2,
            # so the PE never sits on an exp dependency without queued work
            # and the ACT exp stream is continuously fed.
            JB = NT // 2
            stages = [(c, h, jb)
                      for c in range(NC2) for h in range(H) for jb in range(JB)]
            sc_of = {}
            et_of = {}
            pav_of = {}

            def emit_scores(k):
                c, h, jb = stages[k]
                hb = (h % 2) * 64
                ht = h // 2
                if jb == 0:
                    pav_of[(c, h)] = ps_av.tile([128, IS, DH + 1], F32,
                                                tag="av", name="pav")
                ps_sc = ps_s.tile([128, 2, 512], F32, tag="ps", name="ps_sc")
                for e in range(2):
                    jt = jb * 2 + e
                    # scoresT[j, i] = sum_d kT[d, j] qT[d, i]
                    nc.tensor.matmul(
                        ps_sc[:, e, :],
                        qkT[hb:hb + 64, FT + ht, jt * 128:(jt + 1) * 128],
                        qkT[hb:hb + 64, ht, c * 512:(c + 1) * 512],
                    )
                et = exps.tile([128, 2, 512], BF16, tag="exp", name="et")
                nc.scalar.activation(
                    out=et, in_=ps_sc,
                    func=mybir.ActivationFunctionType.Exp,
                    scale=float(SCALE),
                )
                sc_of[k] = ps_sc
                et_of[k] = et

            def emit_av(k):
                c, h, jb = stages[k]
                et = et_of.pop(k)
                pav = pav_of[(c, h)]
                for e in range(2):
                    jt = jb * 2 + e
                    for isub in range(IS):
                        # natural orientation: [128 i, v|1]. start=True zeroes
                        # the whole 2KB psum bank (zero region), so only the
                        # unit's first matmul starts; later isubs' first
                        # writes overwrite pending-zero bytes (has_written).
                        nc.tensor.matmul(
                            pav[:, isub, :],
                            et[:, e, isub * 128:(isub + 1) * 128],
                            vaug[:, jt, h, :],
                            start=(jt == 0 and isub == 0),
                            stop=(jt == NT - 1 and isub == IS - 1),
                        )
                if jb == JB - 1:
                    # normalize: denominators ride the free axis, one per
                    # isub. For chunk 1 the A_hat V slice-add is fused in
                    # (avN is ready long before); chunk 0 normalizes plain
                    # and merges AV in four row-adds before its transposes.
                    pav = pav_of.pop((c, h))
                    recip = small.tile([128, IS], F32, tag="recip")
                    nc.vector.reciprocal(out=recip, in_=pav[:, :, DH:DH + 1])
                    for isub in range(IS):
                        it = c * IS + isub
                        if c == 1:
                            nc.vector.scalar_tensor_tensor(
                                out=yN[:, it, h * DH:(h + 1) * DH],
                                in0=pav[:, isub, 0:DH],
                                scalar=recip[:, isub:isub + 1],
                                in1=avN[:, it, h * DH:(h + 1) * DH],
                                op0=mybir.AluOpType.mult,
                                op1=mybir.AluOpType.add,
                            )
                        else:
                            nc.vector.tensor_scalar_mul(
                                out=yN[:, it, h * DH:(h + 1) * DH],
                                in0=pav[:, isub, 0:DH],
                                scalar1=recip[:, isub:isub + 1],
                            )

            # ---- fill steps: small PE work items woven between stages --
            # Deadlines (PE is in-order; a score emitted at iter k-2 must
            # have its qkT inputs earlier in program order):
            #   v j-tiles 2k,2k+1 before av(stage jb=k)     -> idx <= k
            #   head h's q/k chunks before scores(4h)       -> idx <= 4h-3
            #   (k chunk c covers j-range jb in {2c, 2c+1} only)
            fill = {}

            def tr_group(ft, c):
                for it in range(c * IS, c * IS + IS):
                    trans_unit(ft, it)

            def emit_v2(nt):
                emit_v1(nt)
                emit_v1(nt + 1)

            for i, nt in enumerate((2, 4, 6)):             # dl: av(nt//2)
                fill[i] = lambda nt=nt: emit_v2(nt)
            # q/k chunks at their deadline slots (head h scores at iter 4h-2)
            # so the early iterations stream scores to ACT as fast as possible
            for i, a in ((3, (1, 0)), (4, (5, 0)), (6, (5, 1)),
                         (11, (2, 0)), (12, (6, 0)), (14, (6, 1)),
                         (19, (3, 0)), (20, (7, 0)), (22, (7, 1)),
                         (24, (1, 1)), (44, (2, 1)), (52, (3, 1)),
                         (25, (0, 1))):
                fill[i] = lambda a=a: emit_qk1(*a)
            for i, it in enumerate((26, 27, 28, 29, 30, 31, 34, 35)):
                fill[it] = lambda i=i: ahat_unit(i)  # dl: merge@32, stt@35

            def c0_av_merge(lo, hi):
                # merge A_hat V into chunk-0 rows (after all c0 stt, iter 31)
                for it in range(lo, hi):
                    nc.vector.tensor_add(
                        yN[:, it, :], yN[:, it, :], avN[:, it, :])

            fill[32] = lambda: c0_av_merge(0, 2)
            fill[33] = lambda: c0_av_merge(2, 4)
            for i, ft in zip((36, 38, 40, 43), range(FT)):  # tr c0 (>=34)
                fill[i] = lambda ft=ft: tr_group(ft, 0)
            fill[41] = lambda: tr_group(0, 1)
            fill[48] = lambda: tr_group(1, 1)
            fill[47] = lambda: out_unit(0)
            fill[50] = lambda: out_unit(1)
            fill[56] = lambda: tr_group(2, 1)
            fill[60] = lambda: out_unit(2)
            fill[62] = lambda: out_unit(3)

            # ---- emission ----------------------------------------------
            # pre-phase: only the c0 chunks gate the first scores; the c1
            # k-chunk must still precede scores(2) in PE program order
            emit_qk1(0, 0)      # q heads 0,1 chunk c0
            emit_qk1(4, 0)      # k heads 0,1 j-tiles 0-3
            NS = len(stages)
            emit_scores(0)
            emit_scores(1)
            emit_v1(0)
            emit_v1(1)
            emit_qk1(4, 1)      # k heads 0,1 j-tiles 4-7 (scores(2,3))
            for k in range(NS):
                if k + 2 < NS:
                    emit_scores(k + 2)
                if k in fill:
                    fill[k]()
                emit_av(k)
            # chunk-1 epilogue: all remaining transposes first (tiny on PE,
            # merges pipeline on DVE), then the out projections back-to-back
            for it in range(IS, NT):
                trans_unit(3, it)
            rings = [nc.sync, nc.scalar, nc.sync, nc.scalar]
            for i, it in enumerate(range(IS, NT)):
                out_unit(it, ring=rings[i])

    nc.compile()
    return nc


def _get_program():
    global _PROGRAM
    if _PROGRAM is None:
        _PROGRAM = _build_program()
    return _PROGRAM


def kernel(x, adj, w_qkv, w_out, b_out):
    bf16 = mybir.dt.np(BF16)
    x = np.asarray(x, dtype=np.float32)
    adj = np.asarray(adj, dtype=np.float32)
    w_qkv = np.ascontiguousarray(np.asarray(w_qkv, dtype=np.float32)).astype(bf16)
    w_out = np.ascontiguousarray(np.asarray(w_out, dtype=np.float32)).astype(bf16)
    b_out = np.asarray(b_out, dtype=np.float32).reshape(1, DIM)

    # host-side: binary adjacency A = ceil(adj)+I (EXACT in fp8e4m3) and the
    # degree-normalization vector; the normalized bias A_hat = D^-1/2 A D^-1/2
    # is applied on device in factored form around the fp8 A @ (D^-1/2 V)
    # matmul (DoubleRow), keeping all quantization error on the V side.
    fp8 = mybir.dt.np(FP8)
    A = np.ceil(adj) + np.eye(N, dtype=np.float32)
    dinv = (A.sum(axis=1) ** -0.5).astype(np.float32)
    aT8 = np.ascontiguousarray(A.T).astype(fp8)
    dinvp = np.ascontiguousarray(dinv.reshape(NT, 128).T)

    nc = _get_program()
    in_maps = []
    for b in range(B):
        in_maps.append({
            "xT": np.ascontiguousarray(x[b].T).astype(bf16),
            "wqkv": w_qkv,
            "aT8": aT8,
            "dinvp": dinvp,
            "wout": w_out,
            "bout": b_out,
        })
    global _last_in_maps
    _last_in_maps = in_maps
    res = run_bass_kernel_spmd(nc, in_maps, list(range(B)))
    out = np.stack([res.results[b]["out"] for b in range(B)], axis=0)
    return out.astype(np.float32)


if __name__ == "__main__":
    rng = np.random.default_rng(0)
    x = rng.standard_normal((B, N, DIM), dtype=np.float32)
    adj = (rng.random((N, N), dtype=np.float32) < 0.05).astype(np.float32) * 0.5
    w_qkv = rng.standard_normal((DIM, 3 * F), dtype=np.float32) * DIM ** -0.5
    w_out = rng.standard_normal((F, DIM), dtype=np.float32) * F ** -0.5
    b_out = np.zeros(DIM, dtype=np.float32)
    out = kernel(x=x, adj=adj, w_qkv=w_qkv, w_out=w_out, b_out=b_out)
    print("out", out.shape, out.dtype, np.abs(out).max())
